# revision 2
# baseline (speedup 1.0000x reference)
"""Trainium2 Bass kernel for nn_DiverseLoss (segment_reduce).

Math: with segment ids r = repeat(arange(N_SEG), BS) (aligned 8-row blocks),

    loss = 1 - sqrt( sum_e ||hs[e] - mean[e//BS]||^2 / E )

and per aligned block of BS rows:

    sum_j ||x_j - m||^2 = sum_j ||x_j||^2 - (1/BS) * ||sum_j x_j||^2

so the whole reduction is:  total_sumsq - total_blocksum_sq / BS.

Device strategy (8 cores, data-parallel over rows):
  * Each core streams its 32768x512 f32 shard in 32 "supertiles" of
    1024 rows, laid out [128 partitions x 4096 free] so partition p holds
    the 8 rows of one segment contiguously (16KB/partition contiguous DMA).
  * Block sums via TensorE: 8 accumulating matmuls with a 128x128 identity
    as the stationary operand sum the 8 rows of every segment into one
    PSUM bank [128, 512].
  * sum(x^2): ScalarE activation(Square) with accum_out.  ACT is otherwise
    idle, and at (224 + FD/2)/1.2GHz it has ~2x headroom vs the DMA floor.
    Putting the big elementwise op here (instead of VectorE) matters: a
    fp32 tensor_tensor on DVE runs in 1x perf mode, (4096+151)/0.96GHz =
    4.4us per supertile = ~142us/pass, which together with the PSUM ops
    made the DVE co-critical with the 187.5us HBM floor.  x2 keeps exactly
    two reader engines (PE + ACT).
  * sum(blocksum^2): VectorE only -- copies the PSUM bank to SBUF (one
    PSUM read port) then squares+accumulates.  ~35us/pass, far off the
    critical path.  Keeping ScalarE away from PSUM avoids the ~30us/pass
    ACT-read-of-PSUM penalty while the PE streams into other banks.
  * Per-supertile partials land in [128, 32] SBUF accumulator columns,
    DMA'd out once at the end; the final tiny reduction is float64 on host.
"""

import numpy as np

N_SEG = 32768
BS = 8
E = N_SEG * BS          # 262144 rows
D = 512
N_CORES = 8
R = E // N_CORES        # 32768 rows per core
P = 128
SUPER_ROWS = 1024       # rows per supertile
NS = R // SUPER_ROWS    # 32 supertiles per core
J = SUPER_ROWS // P     # 8 rows (one segment) per partition
FD = J * D              # 4096 f32 free elems per partition

_NC_CACHE = {}


def _build_nc(reps=1, loop_reps=None, x2_engine="act"):
    """reps>1 unrolls the whole pass in-kernel; loop_reps=N wraps the pass in
    a tc.For_i hardware loop instead (constant IRAM footprint, so N can be
    large -- used for low-noise slope timing).  Accumulators are overwritten
    per pass so results are identical for any rep count."""
    import concourse.bacc as bacc
    import concourse.tile as tile
    from concourse import mybir
    from concourse.masks import make_identity

    f32 = mybir.dt.float32
    # Bacc (not plain Bass): its finalize() runs generate_event_semaphores,
    # which splits multi-semaphore waits into standalone event-semaphore
    # instructions — this walrus allows only ONE sync wait per instruction.
    nc = bacc.Bacc()
    hs_in = nc.declare_dram_parameter("hs_shard", [R, D], f32, isOutput=False)
    acc_out = nc.declare_dram_parameter("acc", [P, 2 * NS], f32, isOutput=True)

    hs_v = hs_in[:, :].rearrange("(s p j) d -> s p (j d)", p=P, j=J)

    with tile.TileContext(nc) as tc:
        with (
            tc.tile_pool(name="singles", bufs=1) as singles,
            tc.tile_pool(name="xpool", bufs=6) as xpool,
            tc.tile_pool(name="scratch", bufs=2) as scratch,
            tc.tile_pool(name="psum", bufs=8, space="PSUM") as psum,
        ):
            ident = singles.tile([P, P], f32)
            make_identity(nc, ident)
            acc_x2 = singles.tile([P, NS], f32)
            acc_bs = singles.tile([P, NS], f32)

            def one_pass():
                for s in range(NS):
                    x2 = xpool.tile([P, FD], f32)
                    nc.sync.dma_start(out=x2, in_=hs_v[s])

                    # block sums: 8 accumulating identity matmuls sum the 8
                    # rows of every segment into one PSUM bank [128, 512]
                    ps = psum.tile([P, D], f32)
                    for j in range(J):
                        nc.tensor.matmul(
                            ps,
                            ident,
                            x2[:, j * D : (j + 1) * D],
                            start=(j == 0),
                            stop=(j == J - 1),
                        )

                    # sum of squares of raw rows
                    sq = scratch.tile([P, FD], f32)
                    if x2_engine == "act":
                        nc.scalar.activation(
                            out=sq,
                            in_=x2,
                            func=mybir.ActivationFunctionType.Square,
                            accum_out=acc_x2[:, s : s + 1],
                        )
                    else:
                        nc.vector.scalar_tensor_tensor(
                            out=sq,
                            in0=x2,
                            scalar=1.0,
                            in1=x2,
                            op0=mybir.AluOpType.mult,
                            op1=mybir.AluOpType.mult,
                            accum_out=acc_x2[:, s : s + 1],
                        )

                    # sum of squared block sums, all on VectorE
                    bs_sb = scratch.tile([P, D], f32)
                    nc.vector.tensor_copy(bs_sb, ps)
                    sq_bs = scratch.tile([P, D], f32)
                    nc.vector.scalar_tensor_tensor(
                        out=sq_bs,
                        in0=bs_sb,
                        scalar=1.0,
                        in1=bs_sb,
                        op0=mybir.AluOpType.mult,
                        op1=mybir.AluOpType.mult,
                        accum_out=acc_bs[:, s : s + 1],
                    )

            if loop_reps is not None:
                with tc.For_i(0, loop_reps, 1):
                    one_pass()
            else:
                for _ in range(reps):
                    one_pass()

            nc.sync.dma_start(out=acc_out[:, 0:NS], in_=acc_x2)
            nc.sync.dma_start(out=acc_out[:, NS : 2 * NS], in_=acc_bs)

    # Runs Bacc's legalization pipeline (event-semaphore wait splitting,
    # ldweights wait hoisting, register allocation, ACT table loads).
    nc.finalize()
    return nc


def _get_nc():
    if "nc" not in _NC_CACHE:
        _NC_CACHE["nc"] = _build_nc()
    return _NC_CACHE["nc"]


def _run_device(hs, **kwargs):
    """hs: full [E, D] f32 array. Returns (per-core results, BassKernelResults)."""
    from concourse.bass_utils import run_bass_kernel_spmd

    nc = _get_nc()
    in_maps = [{"hs_shard": hs[c * R : (c + 1) * R]} for c in range(N_CORES)]
    res = run_bass_kernel_spmd(nc, in_maps, list(range(N_CORES)), **kwargs)
    return res


def _combine(results):
    total_sumsq = 0.0
    total_bs2 = 0.0
    for c in range(N_CORES):
        acc = np.asarray(results[c]["acc"], dtype=np.float64)
        total_sumsq += float(acc[:, :NS].sum())
        total_bs2 += float(acc[:, NS:].sum())
    total = total_sumsq - total_bs2 / BS
    return np.asarray(1.0 - np.sqrt(total / E), dtype=np.float32)


def _host_fallback(hs, bsv, edge_index):
    # General (unstructured segment ids) path; exact float64 reference math.
    r = np.asarray(edge_index)[:, 0].astype(np.int64)
    n_seg = hs.shape[0] // bsv
    hs64 = hs.astype(np.float64)
    seg_sum = np.zeros((n_seg, hs.shape[1]), dtype=np.float64)
    np.add.at(seg_sum, r, hs64)
    cnt = np.bincount(r, minlength=n_seg).astype(np.float64)
    mean = seg_sum / np.maximum(cnt, 1.0)[:, None]
    mean_t = np.repeat(mean, bsv, axis=0)
    total = ((hs64 - mean_t) ** 2).sum()
    return np.asarray(1.0 - np.sqrt(total / hs.shape[0]), dtype=np.float32)


def kernel(hs, bs, edge_index):
    hs = np.ascontiguousarray(np.asarray(hs), dtype=np.float32)
    bsv = int(np.asarray(bs))
    ei = np.asarray(edge_index)
    structured = (
        bsv == BS
        and hs.shape == (E, D)
        and np.array_equal(ei[:, 0], np.repeat(np.arange(N_SEG, dtype=ei.dtype), BS))
    )
    if not structured:
        return _host_fallback(hs, bsv, ei)
    res = _run_device(hs)
    return _combine(res.results)


# revision 3
# speedup vs baseline: 1.1705x; 1.1705x over previous
"""Trainium2 Bass kernel for nn_DiverseLoss (segment_reduce).

Math: with segment ids r = repeat(arange(N_SEG), BS) (aligned 8-row blocks),

    loss = 1 - sqrt( sum_e ||hs[e] - mean[e//BS]||^2 / E )

and per aligned block of BS rows:

    sum_j ||x_j - m||^2 = sum_j ||x_j||^2 - (1/BS) * ||sum_j x_j||^2

so the whole reduction is:  total_sumsq - total_blocksum_sq / BS.

Device strategy (8 cores, data-parallel over rows):
  * Each core streams its 32768x512 f32 shard in 32 "supertiles" of
    1024 rows, laid out [128 partitions x 4096 free] so partition p holds
    the 8 rows of one segment contiguously (16KB/partition contiguous DMA).
    The whole supertile is one contiguous 2MiB HBM range -> near-peak DMA
    efficiency; measured DMA-only floor is ~188us/pass, right at the
    358GB/s-per-core HBM limit (64MiB/core).
  * NO TensorEngine.  An earlier version summed the 8 rows per segment via
    8 accumulating identity matmuls; ablation showed DMA+PE alone runs at
    ~262us/pass vs the 188us DMA floor -- the fp32 matmuls (2 cycles/col)
    plus HAM clock-throttle oscillation from the bursty duty cycle make PE
    the buffer-reuse rate limiter.  Instead the per-segment block sum is a
    3-step binary add tree on VectorE (fp32 tensor_tensor is an exact
    (N+151)/0.96GHz: 2048+1024+512 adds = 4.2us/supertile < 5.86us DMA).
  * sum(x^2): ScalarE activation(Square) with accum_out, ~3.6us/supertile.
    ACT's full-size `out` lands in PSUM (free: PE is unused), saving 32KiB
    of SBUF per partition and letting the DMA pool go deeper (bufs=8).
  * sum(blocksum^2): second ACT Square on the [P,512] block sums.
  * x2 has exactly two reader engines (ACT + DVE); every engine runs below
    the DMA cadence, so the stream is HBM-bound end to end.
  * Per-supertile partials land in [128, 32] SBUF accumulator columns,
    DMA'd out once at the end; the final tiny reduction is float64 on host.
"""

import numpy as np

N_SEG = 32768
BS = 8
E = N_SEG * BS          # 262144 rows
D = 512
N_CORES = 8
R = E // N_CORES        # 32768 rows per core
P = 128
SUPER_ROWS = 1024       # rows per supertile
NS = R // SUPER_ROWS    # 32 supertiles per core
J = SUPER_ROWS // P     # 8 rows (one segment) per partition
FD = J * D              # 4096 f32 free elems per partition
XBUFS = 8               # DMA tile pool depth

_NC_CACHE = {}


def _build_nc(reps=1, loop_reps=None):
    """reps>1 unrolls the whole pass in-kernel; loop_reps=N wraps the pass in
    a tc.For_i hardware loop instead (constant IRAM footprint, so N can be
    large -- used for low-noise slope timing).  Accumulators are overwritten
    per pass so results are identical for any rep count."""
    import concourse.bacc as bacc
    import concourse.tile as tile
    from concourse import mybir

    f32 = mybir.dt.float32
    Sq = mybir.ActivationFunctionType.Square
    # Bacc (not plain Bass): its finalize() runs generate_event_semaphores,
    # which splits multi-semaphore waits into standalone event-semaphore
    # instructions — this walrus allows only ONE sync wait per instruction.
    nc = bacc.Bacc()
    hs_in = nc.declare_dram_parameter("hs_shard", [R, D], f32, isOutput=False)
    acc_out = nc.declare_dram_parameter("acc", [P, 2 * NS], f32, isOutput=True)

    hs_v = hs_in[:, :].rearrange("(s p j) d -> s p (j d)", p=P, j=J)

    with tile.TileContext(nc) as tc:
        with (
            tc.tile_pool(name="singles", bufs=1) as singles,
            tc.tile_pool(name="xpool", bufs=XBUFS) as xpool,
            tc.tile_pool(name="scratch", bufs=2) as scratch,
            tc.tile_pool(name="psum", bufs=1, space="PSUM") as psum,
        ):
            acc_x2 = singles.tile([P, NS], f32)
            acc_bs = singles.tile([P, NS], f32)

            def one_pass():
                for s in range(NS):
                    x2 = xpool.tile([P, FD], f32)
                    nc.sync.dma_start(out=x2, in_=hs_v[s])

                    # sum of squares of raw rows on ScalarE; the full-size
                    # mandatory `out` goes to PSUM (PE unused -> PSUM free)
                    sq = psum.tile([P, FD], f32)
                    nc.scalar.activation(
                        out=sq, in_=x2, func=Sq,
                        accum_out=acc_x2[:, s : s + 1],
                    )

                    # per-segment block sums: binary add tree on VectorE.
                    # partition p holds rows j=0..7 of one segment at
                    # [j*512, (j+1)*512); fold 4096 -> 2048 -> 1024 -> 512.
                    t1 = scratch.tile([P, FD // 2], f32)
                    nc.vector.tensor_tensor(
                        out=t1, in0=x2[:, : FD // 2], in1=x2[:, FD // 2 :],
                        op=mybir.AluOpType.add,
                    )
                    t2 = scratch.tile([P, FD // 4], f32)
                    nc.vector.tensor_tensor(
                        out=t2, in0=t1[:, : FD // 4], in1=t1[:, FD // 4 :],
                        op=mybir.AluOpType.add,
                    )
                    bsum = scratch.tile([P, D], f32)
                    nc.vector.tensor_tensor(
                        out=bsum, in0=t2[:, :D], in1=t2[:, D:],
                        op=mybir.AluOpType.add,
                    )

                    # sum of squared block sums on ScalarE
                    sqb = scratch.tile([P, D], f32)
                    nc.scalar.activation(
                        out=sqb, in_=bsum, func=Sq,
                        accum_out=acc_bs[:, s : s + 1],
                    )

            if loop_reps is not None:
                with tc.For_i(0, loop_reps, 1):
                    one_pass()
            else:
                for _ in range(reps):
                    one_pass()

            nc.sync.dma_start(out=acc_out[:, 0:NS], in_=acc_x2)
            nc.sync.dma_start(out=acc_out[:, NS : 2 * NS], in_=acc_bs)

    # Runs Bacc's legalization pipeline (event-semaphore wait splitting,
    # register allocation, ACT table loads).
    nc.finalize()
    return nc


def _get_nc():
    if "nc" not in _NC_CACHE:
        _NC_CACHE["nc"] = _build_nc()
    return _NC_CACHE["nc"]


def _run_device(hs, **kwargs):
    """hs: full [E, D] f32 array. Returns (per-core results, BassKernelResults)."""
    from concourse.bass_utils import run_bass_kernel_spmd

    nc = _get_nc()
    in_maps = [{"hs_shard": hs[c * R : (c + 1) * R]} for c in range(N_CORES)]
    res = run_bass_kernel_spmd(nc, in_maps, list(range(N_CORES)), **kwargs)
    return res


def _combine(results):
    total_sumsq = 0.0
    total_bs2 = 0.0
    for c in range(N_CORES):
        acc = np.asarray(results[c]["acc"], dtype=np.float64)
        total_sumsq += float(acc[:, :NS].sum())
        total_bs2 += float(acc[:, NS:].sum())
    total = total_sumsq - total_bs2 / BS
    return np.asarray(1.0 - np.sqrt(total / E), dtype=np.float32)


def _host_fallback(hs, bsv, edge_index):
    # General (unstructured segment ids) path; exact float64 reference math.
    r = np.asarray(edge_index)[:, 0].astype(np.int64)
    n_seg = hs.shape[0] // bsv
    hs64 = hs.astype(np.float64)
    seg_sum = np.zeros((n_seg, hs.shape[1]), dtype=np.float64)
    np.add.at(seg_sum, r, hs64)
    cnt = np.bincount(r, minlength=n_seg).astype(np.float64)
    mean = seg_sum / np.maximum(cnt, 1.0)[:, None]
    mean_t = np.repeat(mean, bsv, axis=0)
    total = ((hs64 - mean_t) ** 2).sum()
    return np.asarray(1.0 - np.sqrt(total / hs.shape[0]), dtype=np.float32)


def kernel(hs, bs, edge_index):
    hs = np.ascontiguousarray(np.asarray(hs), dtype=np.float32)
    bsv = int(np.asarray(bs))
    ei = np.asarray(edge_index)
    structured = (
        bsv == BS
        and hs.shape == (E, D)
        and np.array_equal(ei[:, 0], np.repeat(np.arange(N_SEG, dtype=ei.dtype), BS))
    )
    if not structured:
        return _host_fallback(hs, bsv, ei)
    res = _run_device(hs)
    return _combine(res.results)


# revision 5
# speedup vs baseline: 1.3362x; 1.1416x over previous
"""Trainium2 Bass kernel for nn_DiverseLoss (segment_reduce).

Math: with segment ids r = repeat(arange(N_SEG), BS) (aligned 8-row blocks),

    loss = 1 - sqrt( sum_e ||hs[e] - mean[e//BS]||^2 / E )

and per aligned block of BS rows:

    sum_j ||x_j - m||^2 = sum_j ||x_j||^2 - (1/BS) * ||sum_j x_j||^2

so the whole reduction is:  total_sumsq - total_blocksum_sq / BS.

Device strategy (8 cores, data-parallel over rows):
  * Each core streams its 32768x512 f32 shard in 32 "supertiles" of
    1024 rows, laid out [128 partitions x 4096 free] so partition p holds
    the 8 rows of one segment contiguously (16KB/partition contiguous DMA).
    The whole supertile is one contiguous 2MiB HBM range -> near-peak DMA
    efficiency; measured DMA-only floor is ~188us/pass, right at the
    358GB/s-per-core HBM limit (64MiB/core).
  * NO TensorEngine.  An earlier version summed the 8 rows per segment via
    8 accumulating identity matmuls; ablation showed DMA+PE alone runs at
    ~262us/pass vs the 188us DMA floor -- the fp32 matmuls (2 cycles/col)
    plus HAM clock-throttle oscillation from the bursty duty cycle make PE
    the buffer-reuse rate limiter.  Instead the per-segment block sum is a
    3-step binary add tree on VectorE (fp32 tensor_tensor is an exact
    (N+151)/0.96GHz: 2048+1024+512 adds = 4.2us/supertile < 5.86us DMA).
  * sum(x^2): ScalarE activation(Square) with accum_out, ~3.6us/supertile.
    ACT's full-size `out` lands in PSUM (free: PE is unused), saving 32KiB
    of SBUF per partition and letting the DMA pool go deeper (bufs=8).
  * sum(blocksum^2): second ACT Square on the [P,512] block sums.
  * x2 has exactly two reader engines (ACT + DVE); every engine runs below
    the DMA cadence, so the stream is HBM-bound end to end.
  * Per-supertile partials land in [128, 32] SBUF accumulator columns,
    DMA'd out once at the end; the final tiny reduction is float64 on host.
"""

import numpy as np

N_SEG = 32768
BS = 8
E = N_SEG * BS          # 262144 rows
D = 512
N_CORES = 8
R = E // N_CORES        # 32768 rows per core
P = 128
SUPER_ROWS = 1024       # rows per supertile
NS = R // SUPER_ROWS    # 32 supertiles per core
J = SUPER_ROWS // P     # 8 rows (one segment) per partition
FD = J * D              # 4096 f32 free elems per partition
XBUFS = 8               # DMA tile pool depth

_NC_CACHE = {}


def _build_nc(reps=1, loop_reps=None, body_passes=1):
    """reps>1 unrolls the whole pass in-kernel; loop_reps=N wraps the pass in
    a tc.For_i hardware loop instead (constant IRAM footprint, so N can be
    large -- used for low-noise slope timing).  body_passes unrolls several
    passes inside each For_i iteration to amortize the ~2us back-edge barrier
    and pipeline drain over more passes.  Accumulators are overwritten per
    pass so results are identical for any rep count."""
    import concourse.bacc as bacc
    import concourse.tile as tile
    from concourse import mybir

    f32 = mybir.dt.float32
    Sq = mybir.ActivationFunctionType.Square
    # Bacc (not plain Bass): its finalize() runs generate_event_semaphores,
    # which splits multi-semaphore waits into standalone event-semaphore
    # instructions — this walrus allows only ONE sync wait per instruction.
    nc = bacc.Bacc()
    hs_in = nc.declare_dram_parameter("hs_shard", [R, D], f32, isOutput=False)
    acc_out = nc.declare_dram_parameter("acc", [P, 2 * NS], f32, isOutput=True)

    hs_v = hs_in[:, :].rearrange("(s p j) d -> s p (j d)", p=P, j=J)

    with tile.TileContext(nc) as tc:
        with (
            tc.tile_pool(name="singles", bufs=1) as singles,
            tc.tile_pool(name="xpool", bufs=XBUFS) as xpool,
            tc.tile_pool(name="scratch", bufs=2) as scratch,
            tc.tile_pool(name="psum", bufs=1, space="PSUM") as psum,
        ):
            acc_x2 = singles.tile([P, NS], f32)
            acc_bs = singles.tile([P, NS], f32)

            def one_pass():
                for s in range(NS):
                    x2 = xpool.tile([P, FD], f32)
                    nc.sync.dma_start(out=x2, in_=hs_v[s])

                    # sum of squares of raw rows on ScalarE; the full-size
                    # mandatory `out` goes to PSUM (PE unused -> PSUM free)
                    sq = psum.tile([P, FD], f32)
                    nc.scalar.activation(
                        out=sq, in_=x2, func=Sq,
                        accum_out=acc_x2[:, s : s + 1],
                    )

                    # per-segment block sums: binary add tree on VectorE.
                    # partition p holds rows j=0..7 of one segment at
                    # [j*512, (j+1)*512); fold 4096 -> 2048 -> 1024 -> 512.
                    t1 = scratch.tile([P, FD // 2], f32)
                    nc.vector.tensor_tensor(
                        out=t1, in0=x2[:, : FD // 2], in1=x2[:, FD // 2 :],
                        op=mybir.AluOpType.add,
                    )
                    t2 = scratch.tile([P, FD // 4], f32)
                    nc.vector.tensor_tensor(
                        out=t2, in0=t1[:, : FD // 4], in1=t1[:, FD // 4 :],
                        op=mybir.AluOpType.add,
                    )
                    bsum = scratch.tile([P, D], f32)
                    nc.vector.tensor_tensor(
                        out=bsum, in0=t2[:, :D], in1=t2[:, D:],
                        op=mybir.AluOpType.add,
                    )

                    # sum of squared block sums on ScalarE
                    sqb = scratch.tile([P, D], f32)
                    nc.scalar.activation(
                        out=sqb, in_=bsum, func=Sq,
                        accum_out=acc_bs[:, s : s + 1],
                    )

            if loop_reps is not None:
                with tc.For_i(0, loop_reps, 1):
                    for _ in range(body_passes):
                        one_pass()
            else:
                for _ in range(reps):
                    one_pass()

            nc.sync.dma_start(out=acc_out[:, 0:NS], in_=acc_x2)
            nc.sync.dma_start(out=acc_out[:, NS : 2 * NS], in_=acc_bs)

    # Runs Bacc's legalization pipeline (event-semaphore wait splitting,
    # register allocation, ACT table loads).
    nc.finalize()
    return nc


def _get_nc():
    if "nc" not in _NC_CACHE:
        _NC_CACHE["nc"] = _build_nc()
    return _NC_CACHE["nc"]


def _run_device(hs, **kwargs):
    """hs: full [E, D] f32 array. Returns (per-core results, BassKernelResults)."""
    from concourse.bass_utils import run_bass_kernel_spmd

    nc = _get_nc()
    in_maps = [{"hs_shard": hs[c * R : (c + 1) * R]} for c in range(N_CORES)]
    res = run_bass_kernel_spmd(nc, in_maps, list(range(N_CORES)), **kwargs)
    return res


def _combine(results):
    total_sumsq = 0.0
    total_bs2 = 0.0
    for c in range(N_CORES):
        acc = np.asarray(results[c]["acc"], dtype=np.float64)
        total_sumsq += float(acc[:, :NS].sum())
        total_bs2 += float(acc[:, NS:].sum())
    total = total_sumsq - total_bs2 / BS
    return np.asarray(1.0 - np.sqrt(total / E), dtype=np.float32)


def _host_fallback(hs, bsv, edge_index):
    # General (unstructured segment ids) path; exact float64 reference math.
    r = np.asarray(edge_index)[:, 0].astype(np.int64)
    n_seg = hs.shape[0] // bsv
    hs64 = hs.astype(np.float64)
    seg_sum = np.zeros((n_seg, hs.shape[1]), dtype=np.float64)
    np.add.at(seg_sum, r, hs64)
    cnt = np.bincount(r, minlength=n_seg).astype(np.float64)
    mean = seg_sum / np.maximum(cnt, 1.0)[:, None]
    mean_t = np.repeat(mean, bsv, axis=0)
    total = ((hs64 - mean_t) ** 2).sum()
    return np.asarray(1.0 - np.sqrt(total / hs.shape[0]), dtype=np.float32)


def kernel(hs, bs, edge_index):
    hs = np.ascontiguousarray(np.asarray(hs), dtype=np.float32)
    bsv = int(np.asarray(bs))
    ei = np.asarray(edge_index)
    structured = (
        bsv == BS
        and hs.shape == (E, D)
        and np.array_equal(ei[:, 0], np.repeat(np.arange(N_SEG, dtype=ei.dtype), BS))
    )
    if not structured:
        return _host_fallback(hs, bsv, ei)
    res = _run_device(hs)
    return _combine(res.results)
